# revision 5
# baseline (speedup 1.0000x reference)
"""MoE DynamicRouter kernel for Trainium2 (8 NeuronCores, SPMD data-parallel).

Math (matches the dense-masked reference):
  router_logits = x @ w_router            [T, E]
  probs = softmax(logits)                 [T, E]
  top-2 combine weights w[t,e] = probs[t,e] * (probs[t,e] >= second_max[t]) / (m1+m2)
  y_e = gelu(x @ w1[e] + b1[e]) @ w2[e] + b2[e]
  out[t] = sum_e w[t,e] * y_e[t]
  aux = 0.01 * sum_e (mean_t probs[t,e] - 1/E)^2

Sharding: data-parallel over the 8192 tokens, 1024 tokens per core; weights
replicated. Everything is computed in a transposed layout (x supplied as
xT=[H,T]) so every matmul contracts over the partition dim and no activation
transposes are needed. Matmuls run in float32r (TF32-like) at full PE rate.
"""

import numpy as np

import concourse.bass as bass
import concourse.mybir as mybir
from concourse import bacc
from concourse.tile import TileContext
from concourse.masks import make_identity
from concourse.bass_utils import run_bass_kernel_spmd

P = 128
B, S, H, E = 4, 2048, 1024, 8
F = 2 * H
NCORES = 8
T_FULL = (B * S) // NCORES  # 1024 tokens per core
HC = H // P  # 8  (h chunks)
FC = F // P  # 16 (f chunks)
f32 = mybir.dt.float32
f32r = mybir.dt.float32r
AF = mybir.ActivationFunctionType
OP = mybir.AluOpType

_BUILD_CACHE = {}


def build(T=T_FULL, TB=512):
    key = (T, TB)
    if key in _BUILD_CACHE:
        return _BUILD_CACHE[key]
    NB = T // TB
    TT = T // P  # token tiles for router
    assert T % TB == 0 and T % P == 0

    nc = bacc.Bacc(None, target_bir_lowering=False, debug=False)

    xT = nc.dram_tensor("xT", [H, T], f32r, kind="ExternalInput")
    wr = nc.dram_tensor("wrp", [P, HC, E], f32r, kind="ExternalInput")  # packed router
    w1 = nc.dram_tensor("w1s", [E, H, F], f32r, kind="ExternalInput")
    b1 = nc.dram_tensor("b1p", [P, E, FC], f32, kind="ExternalInput")  # packed bias1
    w2 = nc.dram_tensor("w2s", [E, F, H], f32r, kind="ExternalInput")
    b2 = nc.dram_tensor("b2s", [E, H], f32r, kind="ExternalInput")
    oh = nc.dram_tensor("onehot", [P, E, P], f32r, kind="ExternalInput")
    zrs = nc.dram_tensor("zeros", [P, 1024], f32r, kind="ExternalInput")
    ons = nc.dram_tensor("ones", [P, 1], f32r, kind="ExternalInput")
    outT = nc.dram_tensor("outT", [H, T], f32, kind="ExternalOutput")
    probsum = nc.dram_tensor("probsum", [1, E], f32, kind="ExternalOutput")

    with TileContext(nc) as tc:
        with (
            tc.tile_pool(name="const", bufs=1) as cpool,
            tc.tile_pool(name="xpool", bufs=1) as xpool,
            tc.tile_pool(name="router", bufs=2) as rpool,
            tc.tile_pool(name="w1pool", bufs=2) as w1pool,
            tc.tile_pool(name="w2pool", bufs=2) as w2pool,
            tc.tile_pool(name="gpool", bufs=1) as gpool,
            tc.tile_pool(name="accpool", bufs=1) as accpool,
            tc.tile_pool(name="wbpool", bufs=2) as wbpool,
            tc.tile_pool(name="gtmp", bufs=3) as gtmppool,
            tc.tile_pool(name="psum1", bufs=2, space="PSUM") as psum1,
            tc.tile_pool(name="psum2", bufs=2, space="PSUM") as psum2,
            tc.tile_pool(name="psmisc", bufs=2, space="PSUM") as psmisc,
            tc.tile_pool(name="psaux", bufs=1, space="PSUM") as psaux,
        ):
            # ---- constants ----
            identity = cpool.tile([P, P], f32)
            make_identity(nc, identity)
            ones_col = cpool.tile([P, 1], f32r)
            nc.sync.dma_start(ones_col, ons[:])
            onehot = cpool.tile([P, E, P], f32r)  # onehot[:, e, :]: row e is ones
            nc.sync.dma_start(onehot, oh[:])

            # ---- resident inputs ----
            xT_sb = xpool.tile([P, HC, T], f32r)
            nc.sync.dma_start(xT_sb, xT.rearrange("(kc p) t -> p kc t", p=P))
            wr_sb = cpool.tile([P, HC, E], f32r)
            nc.sync.dma_start(wr_sb, wr[:])
            b1_sb = cpool.tile([P, E, FC], f32)
            nc.sync.dma_start(b1_sb, b1[:])
            b2_sb = cpool.tile([P, HC, P], f32r)
            nc.sync.dma_start(b2_sb, zrs.rearrange("p (a b) -> p a b", b=P))
            nc.sync.dma_start(
                b2_sb[0:E, :, :], b2.rearrange("e (hc m) -> e hc m", m=P)
            )

            # combine-weights, transposed+zero-padded: rows 0..E-1 valid
            wT_sb = cpool.tile([P, T], f32r)
            nc.sync.dma_start(wT_sb, zrs[:, 0:T])

            ps_aux = psaux.tile([1, E], f32)

            # ================= Stage A: router =================
            for tt in range(TT):
                ps_r = psmisc.tile([P, E], f32, tag="psmisc")
                for kc in range(HC):
                    nc.tensor.matmul(
                        ps_r,
                        xT_sb[:, kc, tt * P : (tt + 1) * P],
                        wr_sb[:, kc, :],
                        start=(kc == 0),
                        stop=(kc == HC - 1),
                    )
                logits = rpool.tile([P, E], f32, tag="logits")
                nc.vector.tensor_copy(logits, ps_r)
                mx = rpool.tile([P, 1], f32, tag="mx")
                nc.vector.tensor_reduce(mx, logits, axis=mybir.AxisListType.X, op=OP.max)
                negmx = rpool.tile([P, 1], f32, tag="negmx")
                nc.vector.tensor_scalar_mul(negmx, mx, -1.0)
                probs_u = rpool.tile([P, E], f32, tag="probs_u")
                nc.scalar.activation(probs_u, logits, AF.Exp, bias=negmx)
                ssum = rpool.tile([P, 1], f32, tag="ssum")
                nc.vector.tensor_reduce(
                    ssum, probs_u, axis=mybir.AxisListType.X, op=OP.add
                )
                rsum = rpool.tile([P, 1], f32, tag="rsum")
                nc.vector.reciprocal(rsum, ssum)
                probs = rpool.tile([P, E], f32, tag="probs")
                nc.vector.tensor_scalar_mul(probs, probs_u, rsum)

                # aux-loss accumulation: column-sum of probs via matmul
                probs_r = rpool.tile([P, E], f32r, tag="probs_r")
                nc.scalar.activation(probs_r, probs, AF.Copy)
                nc.tensor.matmul(
                    ps_aux,
                    ones_col,
                    probs_r,
                    start=(tt == 0),
                    stop=(tt == TT - 1),
                )

                # top-2 combine weights
                max8 = rpool.tile([P, 8], f32, tag="max8")
                nc.vector.max(max8, probs)
                denom = rpool.tile([P, 1], f32, tag="denom")
                nc.vector.tensor_add(denom, max8[:, 0:1], max8[:, 1:2])
                rden = rpool.tile([P, 1], f32, tag="rden")
                nc.vector.reciprocal(rden, denom)
                mask = rpool.tile([P, E], f32, tag="mask")
                nc.vector.tensor_scalar(mask, probs, max8[:, 1:2], None, op0=OP.is_ge)
                wn = rpool.tile([P, E], f32, tag="wn")
                nc.vector.tensor_scalar_mul(wn, probs, rden)
                w_comb = rpool.tile([P, E], f32, tag="w_comb")
                nc.vector.tensor_mul(w_comb, wn, mask)

                # transpose [P, E] -> [E, P] and park in wT_sb
                ps_t = psmisc.tile([P, P], f32, tag="psmisc")
                nc.tensor.transpose(ps_t[0:E, :], w_comb, identity)
                nc.scalar.activation(
                    wT_sb[0:E, tt * P : (tt + 1) * P], ps_t[0:E, :], AF.Copy
                )

            aux_sb = rpool.tile([1, E], f32, tag="aux_sb")
            nc.vector.tensor_copy(aux_sb, ps_aux)
            nc.sync.dma_start(probsum[:], aux_sb)

            # ================= Stage B: experts =================
            acc_sb = accpool.tile([P, HC, T], f32)
            for e in range(E):
                # broadcast combine-weight row e -> [P, T]
                wB_sb = wbpool.tile([P, T], f32, tag="wB")
                for nb in range(NB):
                    ps_b = psmisc.tile([P, TB], f32, tag="psmisc")
                    nc.tensor.matmul(
                        ps_b,
                        onehot[:, e, :],
                        wT_sb[:, nb * TB : (nb + 1) * TB],
                        start=True,
                        stop=True,
                    )
                    nc.vector.tensor_copy(wB_sb[:, nb * TB : (nb + 1) * TB], ps_b)

                # ---- layer 1 + gelu + gate-scale ----
                gp_sb = gpool.tile([P, FC, T], f32r, tag="gp")
                for fcg in range(FC // 2):  # stream w1 in 2-fc chunks
                    w1c = w1pool.tile([P, HC, 2 * P], f32r, tag="w1c")
                    nc.sync.dma_start(
                        w1c,
                        w1[e].rearrange("(kc p) f -> p kc f", p=P)[
                            :, :, fcg * 2 * P : (fcg + 1) * 2 * P
                        ],
                    )
                    for sub in range(2):
                        fc = fcg * 2 + sub
                        for nb in range(NB):
                            ps1 = psum1.tile([P, TB], f32, tag="ps1")
                            for kc in range(HC):
                                nc.tensor.matmul(
                                    ps1,
                                    w1c[:, kc, sub * P : (sub + 1) * P],
                                    xT_sb[:, kc, nb * TB : (nb + 1) * TB],
                                    start=(kc == 0),
                                    stop=(kc == HC - 1),
                                )
                            gt = gtmppool.tile([P, TB], f32, tag="gt")
                            nc.scalar.activation(
                                gt, ps1, AF.Gelu, bias=b1_sb[:, e, fc : fc + 1]
                            )
                            nc.vector.tensor_mul(
                                gp_sb[:, fc, nb * TB : (nb + 1) * TB],
                                gt,
                                wB_sb[:, nb * TB : (nb + 1) * TB],
                            )

                # ---- layer 2 + accumulate over experts ----
                for hc in range(HC):
                    w2c = w2pool.tile([P, FC, P], f32r, tag="w2c")
                    nc.sync.dma_start(
                        w2c,
                        w2[e].rearrange("(fc p) h -> p fc h", p=P)[
                            :, :, hc * P : (hc + 1) * P
                        ],
                    )
                    for nb in range(NB):
                        ps2 = psum2.tile([P, TB], f32, tag="ps2")
                        for fc in range(FC):
                            nc.tensor.matmul(
                                ps2,
                                w2c[:, fc, :],
                                gp_sb[:, fc, nb * TB : (nb + 1) * TB],
                                start=(fc == 0),
                                stop=(fc == FC - 1 and e != 0),
                            )
                        if e == 0:
                            # bias2 contribution: sum_e b2[e,h] * w[t,e]
                            nc.tensor.matmul(
                                ps2,
                                b2_sb[:, hc, :],
                                wT_sb[:, nb * TB : (nb + 1) * TB],
                                start=False,
                                stop=True,
                            )
                            nc.vector.tensor_copy(
                                acc_sb[:, hc, nb * TB : (nb + 1) * TB], ps2
                            )
                        else:
                            nc.vector.tensor_add(
                                acc_sb[:, hc, nb * TB : (nb + 1) * TB],
                                acc_sb[:, hc, nb * TB : (nb + 1) * TB],
                                ps2,
                            )

            for hc in range(HC):
                nc.sync.dma_start(outT[hc * P : (hc + 1) * P, :], acc_sb[:, hc, :])

    nc.compile()
    _BUILD_CACHE[key] = nc
    return nc


def _onehot_const():
    oh = np.zeros((P, E, P), dtype=np.float32)
    for e in range(E):
        oh[e, e, :] = 1.0
    return oh


def prep_in_maps(x, w_router, w1, b1, w2, b2, T=T_FULL, ncores=NCORES):
    """Shard inputs for the SPMD kernel. x: [B,S,H] (or [ntok,H])."""
    xflat = np.ascontiguousarray(x, dtype=np.float32).reshape(-1, H)
    w_router = np.ascontiguousarray(w_router, dtype=np.float32)
    wrp = np.ascontiguousarray(w_router.reshape(HC, P, E).transpose(1, 0, 2))
    w1s = np.ascontiguousarray(w1, dtype=np.float32)
    w2s = np.ascontiguousarray(w2, dtype=np.float32)
    b1p = np.ascontiguousarray(
        np.asarray(b1, dtype=np.float32).reshape(E, FC, P).transpose(2, 0, 1)
    )
    b2s = np.ascontiguousarray(b2, dtype=np.float32)
    in_maps = []
    for c in range(ncores):
        shard = xflat[c * T : (c + 1) * T, :]
        in_maps.append(
            {
                "xT": np.ascontiguousarray(shard.T),
                "wrp": wrp,
                "w1s": w1s,
                "b1p": b1p,
                "w2s": w2s,
                "b2s": b2s,
                "onehot": _onehot_const(),
                "zeros": np.zeros((P, 1024), dtype=np.float32),
                "ones": np.ones((P, 1), dtype=np.float32),
            }
        )
    return in_maps


def postprocess(results, T=T_FULL, ncores=NCORES, out_shape=(B, S, H)):
    outs = [np.asarray(r["outT"]).T for r in results]
    output = np.concatenate(outs, axis=0).reshape(*out_shape)
    colsum = np.sum([np.asarray(r["probsum"])[0] for r in results], axis=0)
    usage = colsum / float(T * ncores)
    aux = np.float32(0.01 * np.sum((usage - 1.0 / E) ** 2))
    return output, aux


def kernel(x, w_router, w1, b1, w2, b2):
    nc = build()
    in_maps = prep_in_maps(x, w_router, w1, b1, w2, b2)
    res = run_bass_kernel_spmd(nc, in_maps, core_ids=list(range(NCORES)))
    return postprocess(res.results)
